# revision 32
# baseline (speedup 1.0000x reference)
"""Causal self-attention (N=2, S=4096, E=768, H=12) on 8 NeuronCores.

Sharding: batch x head-group. Core c handles batch n = c // 4 and heads
h0 = (c % 4) * 3 .. h0+2 (3 heads per core, 24 (n,h) pairs over 8 cores).

Matmul operands are bf16 (host converts inputs; fp32 PSUM accumulate),
except the PV (attention-weights x values) matmul for full-width chunk
pairs, which runs in fp8e4m3 with perf_mode=DoubleRow: two key-chunks
per pass through the PE array. exp is shifted by a uniform -2 bias
(cancels in the softmax ratio) so fp8 et stays at/below e^4, half the
e4m3 max. Diagonal chunks keep bf16 PV (largest attention weights).
Measured end-to-end rel err 1.13e-2 on HW vs the 2e-2 gate. PSUM bank
limit
keeps matmul free-dim at 512 fp32, so the slab stays 512; bf16 buys:
half the SBUF/DMA bytes, half the LDWEIGHTS time, full-rate matmuls at
any free-dim (no fp32r >=256 clamp) so diagonal chunks skip their
fully-masked prefix exactly and V projection runs at its natural N=192.

Per-core kernel (SPMD, identical program, per-core input values):
  inputs:  xT   [768, 4096] bf16   x[n] transposed (host layout prep)
           wqk  [3, 768, 128] bf16 per head [Wq_h | Wk_h] column blocks
           wv   [768, 192] bf16    Wv columns for the 3 heads
           bqk  [128, 3] f32       per head [bq_h; bk_h]
           bv   [1, 192] f32
  output:  outT [3, 64, 4096] f32  per-head attention output, transposed

Phase 1 (projections, whole sequence resident in SBUF; xt loaded once):
  per (head, 512-slab): one PSUM tile accumulates [q|k] over 6
  contraction chunks; DVE adds bias, q -> qt[h] (partitions 0-63, bf16),
  k -> kt[h] rows 64-127, then one SBUF DMA drops k to partitions 0-63
  so score matmuls contract on a matching partition range. V: per
  128-seq chunk, PSUM [seq, 192] bias-added into v_aug [128, chunk,
  head, 1+64] whose column 0 is 1.0 so the PV matmul accumulates the
  softmax row-sum into PSUM partition 0 for free.

Phase 2 (flash-style attention, scores never leave the chip):
  scores computed TRANSPOSED: sT[sk-chunk 128, sq-slab 512] =
  matmul(lhsT=kT-chunk, rhs=qT-slab). Key chunks are packed into
  1024-wide PSUM groups (2 full chunks, or the slab's 4 diagonal chunks
  left-shifted to their valid suffix as [512,384,128]+[256] -- chunk
  starts never cross a PSUM bank), so EVERY group is exp'd by one
  ScalarE ACTIVATE (scale=1/8 folded into its free affine, bf16 out,
  zero wasted columns); causal mask zeroed in-place by GPSIMD
  affine_select on each diagonal chunk's leading 128 columns; the PV
  matmul consumes exp(sT) directly as the streaming operand. The
  fully-masked prefix of a diagonal chunk is never computed, exp'd, or
  streamed (chunk 0 of each slab is full-width so PV's start=True
  initializes the whole bank). Division by row-sum: DVE copy
  PSUM->SBUF, DVE reciprocal on partition 0, GPSIMD
  partition_broadcast, DVE multiply -- deferred to slab end and batched
  per stage so the in-order DVE queue never blocks.

  Emission is a lookahead-2 software pipeline over triple-buffered
  score PSUM (12KB) + double-buffered PV accumulators (4KB = exactly
  the 8 PSUM banks): two score groups are always queued on the PE
  ahead of each pv group, so the exp latency (~1.15us + 2 semaphore
  hops) never stalls the in-order PE queue. Projection units for slab
  j+1 are spliced evenly into attention slab j's unit stream (proj(0)
  runs up front, head A's dependencies first), keeping ScalarE fed from
  ~3us onward.

  xt slabs 1..7 are DMA'd just-in-time (one slab ahead of the
  attention slab whose proj splice consumes them) instead of upfront --
  5.4MB issued at t=0 hogged the DMA bandwidth pool for ~15us; the two
  big v_aug/v8 zero-pad memsets run on the otherwise-idle GPSIMD so
  the DVE queue opens with the first projection bias-copies.

  TimelineSim: 251.3us per core (fp32r baseline: 286.0us). ScalarE is
  the modeled wall (210us busy; exp elements + 293ns/instr fixed, the
  instruction count pinned by the PSUM pincer: (lookahead+1) x group
  slots + PV accumulators <= 8 banks). PE dropped to 165us after the
  fp8 DoubleRow PV. HW slope (reps=1-vs-17 interleaved median):
  ~185us/rep, rel err 1.13e-2, vs ~216-231us for the all-bf16 variant
  and ~2469us graded baseline.
"""

import os
import sys

import numpy as np

for _p in ("/opt/trn_rl_repo",):
    if _p not in sys.path and os.path.isdir(_p):
        sys.path.insert(0, _p)

import ml_dtypes  # noqa: E402

import concourse.bass as bass  # noqa: E402
import concourse.mybir as mybir  # noqa: E402
import concourse.tile as tile  # noqa: E402
from concourse import bacc  # noqa: E402

F32 = mybir.dt.float32
BF16 = mybir.dt.bfloat16
FP8 = mybir.dt.float8e4
NP_BF16 = ml_dtypes.bfloat16

N, S, E, H = 2, 4096, 768, 12
D = 64
HPC = 3  # heads per core
P = 128
SLAB = 512
CHUNK = 128
GROUP = 2  # score chunks per exp batch (sc psum tile = GROUP*SLAB)
KCH = E // P  # 6 contraction chunks
VW = HPC * D  # 192 value columns per core


def build_nc(seq=S, n_cores=8, reps=1, abl=()):
    nslab = seq // SLAB
    nchunk = seq // CHUNK
    cps = SLAB // CHUNK  # chunks per slab (4)

    nc = bacc.Bacc("TRN2", target_bir_lowering=False, debug=False,
                   num_devices=n_cores)

    xT_d = nc.dram_tensor("xT", [E, seq], BF16, kind="ExternalInput")
    wqk_d = nc.dram_tensor("wqk", [HPC, E, P], BF16, kind="ExternalInput")
    wv_d = nc.dram_tensor("wv", [E, VW], BF16, kind="ExternalInput")
    bqk_d = nc.dram_tensor("bqk", [P, HPC], F32, kind="ExternalInput")
    bv_d = nc.dram_tensor("bv", [1, VW], F32, kind="ExternalInput")
    outT_d = nc.dram_tensor("outT", [HPC, D, seq], F32, kind="ExternalOutput")

    xT_r = xT_d.ap().rearrange("(o p) s -> p o s", p=P)
    wqk_r = wqk_d.ap().rearrange("h (o p) m -> p h o m", p=P)
    wv_r = wv_d.ap().rearrange("(o p) m -> p o m", p=P)

    add = mybir.AluOpType.add
    mult = mybir.AluOpType.mult
    Exp = mybir.ActivationFunctionType.Exp

    with tile.TileContext(nc) as tc:
        with (
            tc.tile_pool(name="const", bufs=1) as cpool,
            tc.tile_pool(name="persist", bufs=1) as ppool,
            tc.tile_pool(name="ework", bufs=6) as epool,
            tc.tile_pool(name="small", bufs=3) as spool,
            tc.tile_pool(name="psum", bufs=2, space="PSUM") as psum,
        ):
            # ---- constants; wqk + the first x slab lead the DMA queue
            # (they gate the first projection matmul) ----
            wqk_sb = cpool.tile([P, HPC, KCH, P], BF16)
            for _b in range(HPC):
                nc.sync.dma_start(wqk_sb[:, _b], wqk_r[:, _b])
            xt = ppool.tile([P, KCH, seq], BF16, name="xt")
            for _k in range(KCH):
                nc.sync.dma_start(xt[:, _k, 0:SLAB], xT_r[:, _k, 0:SLAB])
            wv_sb = cpool.tile([P, KCH, VW], BF16)
            nc.sync.dma_start(wv_sb[:], wv_r)
            bqk_sb = cpool.tile([P, HPC], F32)
            nc.sync.dma_start(bqk_sb[:], bqk_d.ap())
            bv1_sb = cpool.tile([1, VW], F32)
            nc.sync.dma_start(bv1_sb[:], bv_d.ap())
            # xt slabs 1..7 are DMA'd just-in-time, one slab ahead of the
            # attention slab whose proj splice consumes them -- issuing all
            # 5.4MB upfront hogs the DMA bandwidth pool for ~15us.
            bv_bc = cpool.tile([P, VW], F32)
            nc.gpsimd.partition_broadcast(bv_bc[:], bv1_sb[:])

            # dummy exp: forces the ACT table load at t=0, in parallel
            # with the input DMAs, off the critical path.
            warm = cpool.tile([1, 1], F32)
            nc.vector.memset(warm[:], 0.0)
            nc.scalar.activation(warm[:], warm[:], Exp)
            # uniform exp shift: exp(s/8 - 2) keeps fp8 et <= e^4 (half the
            # e4m3 max); the factor e^-2 cancels in the softmax ratio.
            eb = cpool.tile([P, 1], F32)
            nc.vector.memset(eb[:], -2.0)

            # [1 | 0*63 | v] augmented values: col 0 carries the softmax
            # row-sum (PSUM partition 0, where partition_broadcast sources);
            # v sits at cols 64-127 so the divide chain's DVE ops run on
            # the partition range [64, 128) (64-aligned, DVE-legal).
            v_aug = cpool.tile([P, nchunk, HPC, P], BF16)
            nc.vector.memset(v_aug[:, :, :, 0:1], 1.0)
            # the big zero-pad memsets run on the otherwise-idle GPSIMD:
            # on DVE they monopolize the queue for ~13us and stall the
            # first projection bias-copy (and with it the first exp).
            nc.gpsimd.memset(v_aug[:, :, :, 1:D], 0.0)
            # fp8 copy of the augmented values: NON-diagonal chunk pairs run
            # the PV matmul in fp8e4m3 DoubleRow (2 key-chunks per pass).
            v8 = cpool.tile([P, nchunk, HPC, P], FP8)
            nc.vector.memset(v8[:, :, :, 0:1], 1.0)
            nc.gpsimd.memset(v8[:, :, :, 1:D], 0.0)

            qt = []
            kt = []
            for h in range(HPC):
                qt.append(ppool.tile([D, seq], BF16, name=f"qt{h}"))
                kt.append(ppool.tile([P, seq], BF16, name=f"kt{h}"))

            def proj_qk(j, h):
                if "noproj" in abl:
                    return
                sl = slice(j * SLAB, (j + 1) * SLAB)
                ps = psum.tile([P, SLAB], F32, tag="pv", name="ps")
                for k in range(KCH):
                    nc.tensor.matmul(
                        ps[:],
                        lhsT=wqk_sb[:, h, k, :],
                        rhs=xt[:, k, sl],
                        start=(k == 0),
                        stop=(k == KCH - 1),
                    )
                nc.vector.tensor_scalar_add(
                    qt[h][:, sl], ps[0:D, :], bqk_sb[0:D, h : h + 1]
                )
                nc.vector.tensor_scalar_add(
                    kt[h][D:P, sl], ps[D:P, :], bqk_sb[D:P, h : h + 1]
                )
                nc.sync.dma_start(kt[h][0:D, sl], kt[h][D:P, sl])

            def proj_v(c):
                if "noproj" in abl:
                    return
                vp = psum.tile([P, SLAB], F32, tag="pv", name="vp")
                for k in range(KCH):
                    nc.tensor.matmul(
                        vp[:, 0:VW],
                        lhsT=xt[:, k, c * CHUNK : (c + 1) * CHUNK],
                        rhs=wv_sb[:, k, :],
                        start=(k == 0),
                        stop=(k == KCH - 1),
                    )
                nc.vector.tensor_tensor(
                    v_aug[:, c, :, D : 2 * D],
                    vp[:, 0:VW].rearrange("p (h d) -> p h d", h=HPC),
                    bv_bc[:].rearrange("p (h d) -> p h d", h=HPC),
                    add,
                )
                nc.vector.tensor_tensor(
                    v8[:, c, :, D : 2 * D],
                    vp[:, 0:VW].rearrange("p (h d) -> p h d", h=HPC),
                    bv_bc[:].rearrange("p (h d) -> p h d", h=HPC),
                    add,
                )

            def proj_units(j, head_first=False):
                qks = [(lambda h=h: proj_qk(j, h)) for h in range(HPC)]
                vs = [(lambda c=c: proj_v(c))
                      for c in range(j * cps, (j + 1) * cps)]
                if head_first:
                    # head A's dependencies first so attn(0, A) can start
                    # while B/C projections still stream.
                    return [qks[0]] + vs + qks[1:]
                return qks + vs

            def proj_slab(j, head_first=False):
                for u in proj_units(j, head_first):
                    u()

            def attn_units(h, j):
                """(scores_fn, pv_fn, tail_fn|None) triples for one head's
                slab; emission pipelined across heads by the caller."""
                sl = slice(j * SLAB, (j + 1) * SLAB)
                nch = (j + 1) * cps  # causal: key chunks 0 .. (j+1)*cps-1
                state = {}

                def lo_of(ci):
                    m = ci - j * cps
                    return CHUNK * m if m >= 1 else 0

                # groups: non-diagonal chunks in aligned 2x512 batches; the
                # 4 diagonal chunks form two groups packed [512,384,128]
                # (offsets 0/512/896, exactly 1024, no PSUM bank crossing)
                # and [256], so EVERY group is exp'd by ONE ScalarE
                # instruction with zero wasted columns.
                ndg = j * cps
                groups = [list(range(g * GROUP, min(ndg, (g + 1) * GROUP)))
                          for g in range((ndg + GROUP - 1) // GROUP)]
                groups.append([ndg, ndg + 1, ndg + 3])
                groups.append([ndg + 2])
                ngrp = len(groups)

                def scores_group(g):
                    chunks = groups[g]
                    offs = []
                    off = 0
                    for ci in chunks:
                        offs.append(off)
                        w = SLAB - lo_of(ci)
                        assert off // SLAB == (off + w - 1) // SLAB
                        off += w
                    width = off
                    sc = psum.tile([P, GROUP * SLAB], F32, tag="sc", name="sc",
                                   bufs=3)
                    for ci, off in zip(chunks, offs):
                        lo = lo_of(ci)
                        if "nosc" in abl:
                            continue
                        nc.tensor.matmul(
                            sc[:, off : off + SLAB - lo],
                            lhsT=kt[h][0:D, ci * CHUNK : (ci + 1) * CHUNK],
                            rhs=qt[h][:, j * SLAB + lo : (j + 1) * SLAB],
                            start=True,
                            stop=True,
                        )
                    is_pair = len(chunks) == 2
                    if is_pair:
                        et = epool.tile([P, GROUP * SLAB], FP8, tag="E8",
                                        name="et8", bufs=6)
                    else:
                        et = epool.tile([P, GROUP * SLAB], BF16, tag="E",
                                        name="et")
                    if "noexp" in abl:
                        nc.scalar.activation(et[:, 0:1], sc[:, 0:1], Exp,
                                             scale=0.125, bias=eb[:, 0:1])
                    else:
                        nc.scalar.activation(
                            et[:, :width], sc[:, :width], Exp, scale=0.125,
                            bias=eb[:, 0:1],
                        )
                    for ci, off in zip(chunks, offs):
                        m = ci - j * cps
                        if "noaffine" in abl:
                            continue
                        if 0 <= m < cps:  # triangle: zero sq < sk entries
                            # the chunk's shifted origin IS its diagonal
                            # 128-block (queries 128m.. vs keys of chunk m).
                            nc.gpsimd.affine_select(
                                out=et[:, off : off + CHUNK],
                                in_=et[:, off : off + CHUNK],
                                compare_op=mybir.AluOpType.is_ge,
                                fill=0.0,
                                base=0,
                                pattern=[[1, CHUNK]],
                                channel_multiplier=-1,
                            )
                    state[g] = (et, chunks, offs)

                def pv_group(g):
                    if g == 0:
                        state["pv"] = psum.tile([P, SLAB], F32, tag="pv",
                                                name="pv")
                    pv = state["pv"]
                    et, chunks, offs = state.pop(g)
                    if "nopv" in abl:
                        return
                    if len(chunks) == 2:
                        # fp8e4m3 DoubleRow: both key-chunks of the pair in
                        # one pass. lhsT [128, 2, 128], rhs [128, 2, 512]
                        # (pair dim at index 1), out [128, 512].
                        nc.tensor.matmul(
                            pv[:, 0:SLAB],
                            lhsT=v8[:, chunks[0] : chunks[0] + 2, h, :],
                            rhs=et[:, 0 : 2 * SLAB].rearrange(
                                "p (h2 c) -> p h2 c", h2=2),
                            start=(g == 0),
                            stop=False,
                            perf_mode=mybir.MatmulPerfMode.DoubleRow,
                            skip_group_check=True,
                        )
                        return
                    for ci, off in zip(chunks, offs):
                        lo = lo_of(ci)
                        nc.tensor.matmul(
                            pv[:, lo:SLAB],
                            lhsT=v_aug[:, ci, h, :],
                            rhs=et[:, off : off + SLAB - lo],
                            start=(g == 0 and ci == chunks[0]),
                            stop=(g == ngrp - 1 and ci == chunks[-1]),
                            skip_group_check=True,
                        )

                def cp_fn():
                    if "nodiv" in abl:
                        return
                    # one DVE copy frees the PV psum bank; divide chain
                    # runs from SBUF at slab end.
                    pv = state["pv"]
                    cp = spool.tile([P, SLAB], F32, tag="cp", name="cp")
                    nc.vector.tensor_copy(cp[:], pv[:])
                    state["cp"] = cp

                def recip_fn():
                    if "nodiv" in abl:
                        return
                    cp = state["cp"]
                    nc.vector.reciprocal(cp[0:1, :], cp[0:1, :])

                def rbc_fn():
                    if "nodiv" in abl:
                        return
                    rbc = spool.tile([P, SLAB], F32, tag="rbc",
                                     name="rbc")
                    nc.gpsimd.partition_broadcast(rbc[:], state["cp"][0:1, :])
                    state["rbc"] = rbc

                def mult_fn():
                    if "nodiv" in abl:
                        return
                    osb = spool.tile([P, SLAB], F32, tag="osb",
                                     name="osb")
                    nc.vector.tensor_tensor(
                        osb[D : 2 * D, :],
                        state["cp"][D : 2 * D, :],
                        state["rbc"][D : 2 * D, :],
                        mult,
                    )
                    nc.sync.dma_start(outT_d.ap()[h, :, sl],
                                      osb[D : 2 * D, :])

                units = [
                    [
                        (lambda g=g: scores_group(g)),
                        (lambda g=g: pv_group(g)),
                        None,
                    ]
                    for g in range(ngrp)
                ]
                units[-1][2] = cp_fn
                return units, (recip_fn, rbc_fn, mult_fn)

            def noop():
                return None

            for _rep in range(reps):
                # proj(0) up front; proj(j+1) units are spliced into
                # attn(j)'s pipeline: attention slab j only needs
                # projections of slabs <= j, and the interleave keeps the
                # PE streaming while ScalarE exps attn groups.
                proj_slab(0, head_first=True)
                for j in range(nslab):
                    if _rep == 0 and j + 1 < nslab:
                        _sl = slice((j + 1) * SLAB, (j + 2) * SLAB)
                        nc.sync.dma_start(xt[:, :, _sl], xT_r[:, :, _sl])
                    per_head = []
                    divs = []
                    for h in range(HPC):
                        u, dv = attn_units(h, j)
                        per_head.append(u)
                        divs.append(dv)
                    units = []
                    for u in per_head:
                        units.extend(u)
                    # splice proj(j+1) units evenly into the stream
                    if j + 1 < nslab:
                        pus = proj_units(j + 1)
                        step = max(1, len(units) // len(pus))
                        out = []
                        pi = 0
                        for i, u in enumerate(units):
                            out.append(u)
                            if i % step == step - 1 and pi < len(pus):
                                out.append([pus[pi], noop, None])
                                pi += 1
                        for p in pus[pi:]:
                            out.append([p, noop, None])
                        units = out
                    # lookahead-2 software pipeline: two score groups are
                    # always queued on the PE ahead of each pv group, so
                    # the exp (ScalarE) latency + sem hops never stall the
                    # in-order PE queue. sc psum is triple-buffered.
                    units[0][0]()
                    if len(units) > 1:
                        units[1][0]()
                    for i, (_, pv_f, cp_f) in enumerate(units):
                        if i + 2 < len(units):
                            units[i + 2][0]()
                        pv_f()
                        if cp_f is not None:
                            cp_f()
                    # deferred division chains, batched per stage
                    for stage in range(3):
                        for dv in divs:
                            dv[stage]()

    nc.compile()
    return nc


def shard_inputs(x, Wq, bq, Wk, bk, Wv, bv, n_cores=8, hpc=HPC):
    """Host-side layout prep: slice per-core head groups + transpose x."""
    in_maps = []
    nb = x.shape[0]
    groups = n_cores // nb  # head groups per batch
    xT = [np.ascontiguousarray(x[n].T).astype(NP_BF16) for n in range(nb)]
    for core in range(n_cores):
        n = core // groups
        h0 = (core % groups) * hpc
        wqk = np.stack(
            [
                np.concatenate(
                    [
                        Wq[:, (h0 + i) * D : (h0 + i + 1) * D],
                        Wk[:, (h0 + i) * D : (h0 + i + 1) * D],
                    ],
                    axis=1,
                )
                for i in range(hpc)
            ]
        ).astype(NP_BF16)
        bqk = np.stack(
            [
                np.concatenate(
                    [bq[(h0 + i) * D : (h0 + i + 1) * D],
                     bk[(h0 + i) * D : (h0 + i + 1) * D]]
                )
                for i in range(hpc)
            ],
            axis=1,
        ).astype(np.float32)
        in_maps.append(
            {
                "xT": xT[n],
                "wqk": np.ascontiguousarray(wqk),
                "wv": np.ascontiguousarray(
                    Wv[:, h0 * D : (h0 + hpc) * D]
                ).astype(NP_BF16),
                "bqk": np.ascontiguousarray(bqk),
                "bv": np.ascontiguousarray(
                    bv[None, h0 * D : (h0 + hpc) * D].astype(np.float32)
                ),
            }
        )
    return in_maps


def gather_output(results, n_cores=8, nb=N, seq=S, emb=E, hpc=HPC):
    out = np.empty((nb, seq, emb), np.float32)
    groups = n_cores // nb
    for core in range(n_cores):
        n = core // groups
        h0 = (core % groups) * hpc
        oT = results[core]["outT"]  # [hpc, D, seq]
        for i in range(hpc):
            out[n, :, (h0 + i) * D : (h0 + i + 1) * D] = oT[i].T
    return out


_NC_CACHE = {}


def _get_nc():
    if "nc" not in _NC_CACHE:
        _NC_CACHE["nc"] = build_nc()
    return _NC_CACHE["nc"]


def run_on_hw(inputs, trace=False):
    """Run on the 8 NeuronCores; returns (full_output, BassKernelResults)."""
    from concourse.bass_utils import run_bass_kernel_spmd

    nc = _get_nc()
    in_maps = shard_inputs(**inputs)
    res = run_bass_kernel_spmd(nc, in_maps, list(range(8)), trace=trace)
    return gather_output(res.results), res


def kernel(x, Wq, bq, Wk, bk, Wv, bv):
    x = np.asarray(x)
    out, _ = run_on_hw(
        dict(x=x, Wq=np.asarray(Wq), bq=np.asarray(bq), Wk=np.asarray(Wk),
             bk=np.asarray(bk), Wv=np.asarray(Wv), bv=np.asarray(bv))
    )
    return out.astype(np.float32)
